# revision 1
# baseline (speedup 1.0000x reference)
"""Trainium2 Bass kernel for: conv2d(3x3, VALID) + bias -> channel-min -> tanh(tanh).

Problem shapes (fixed):
  x      [32, 64, 128, 128] f32   (N, C_in, H, W)
  weight [128, 64, 3, 3]    f32   (C_out, C_in, kh, kw)
  bias   [128]              f32
  out    [32, 1, 126, 126]  f32

Strategy
--------
Data-parallel over 8 cores: 4 images per core, weights/bias replicated.

Per core, per image (matmuls in fp16, PSUM accumulation in f32):
  * Two "dup" SBUF tiles per image hold the image twice with a shift, so a
    single K=128 matmul covers two conv taps (C_in=64 channels each):
      D1[0:64, f] = x[c, f]   D1[64:128, f] = x[c, f+1]    (shift 1 px in W)
      DR[0:64, f] = x[c, f]   DR[64:128, f] = x[c, f+128]  (shift 1 row in H)
  * conv for a 4-row output tile (504 px) = 5 accumulating K=128 matmuls:
      3 pairs (kh,0)+(kh,1) via D1, 1 pair (0,2)+(1,2) via DR,
      1 single (2,2) with zero-padded upper weight rows.
    (All matmuls use full K=128 / tile_position (0,0) — mixing row-group
    matmuls inside one accumulation group crashes the NEFF at runtime.)
  * ScalarE applies tanh(y + bias) while copying PSUM -> SBUF fp16.
    (min over channels commutes with the monotone tanh.)
  * PE transposes 128-px chunks so channels land on the free dim, VectorE
    reduce_min over channels -> per-pixel channel-min.
  * Per image, results are collected into O[128, 128], transposed once more
    so pixels are contiguous in the free dim, second tanh on ScalarE, DMA out.

Output tiling: 32 tiles of 4 rows (h0 = 0,4,...,120 and 122 — the last tile
overlaps by 2 rows so every tile is full). Within a tile the 504 px are
covered by 4 chunks starting at 0/128/256/376 (last overlaps by 8 px).
"""

import numpy as np

import concourse.bacc as bacc
import concourse.bass as bass
import concourse.tile as tile
from concourse import mybir
from concourse.bass_utils import run_bass_kernel_spmd

N_CORES = 8
N_IMGS = 32
IMGS_PER_CORE = N_IMGS // N_CORES
C_IN = 64
C_OUT = 128
H = W = 128
HO = WO = 126
NPIX = HO * WO  # 15876
R = 4  # output rows per tile
TILE_H0S = list(range(0, 121, 4)) + [124]  # 31 R=4 tiles + one R=2 tail tile
CHUNK_STARTS = [0, 128, 256, 376]  # pixel chunk starts within a tile
F16 = mybir.dt.float16
F32 = mybir.dt.float32


def build_kernel(reps=1):
    """reps > 1 repeats the whole per-core compute in one NEFF (for HW timing)."""
    nc = bacc.Bacc(trn_type="TRN2", target_bir_lowering=False, debug=False)
    x1 = nc.dram_tensor("x1", [IMGS_PER_CORE, 128, H * W], F16, kind="ExternalInput")
    xr = nc.dram_tensor("xr", [IMGS_PER_CORE, 128, H * W], F16, kind="ExternalInput")
    wp = nc.dram_tensor("wp", [128, 5, 128], F16, kind="ExternalInput")
    bias = nc.dram_tensor("bias", [128, 1], F32, kind="ExternalInput")
    ident = nc.dram_tensor("ident", [128, 128], F16, kind="ExternalInput")
    out = nc.dram_tensor("out", [IMGS_PER_CORE, NPIX], F32, kind="ExternalOutput")

    with tile.TileContext(nc) as tc:
        with (
            tc.tile_pool(name="consts", bufs=1) as consts,
            tc.tile_pool(name="dpool", bufs=2) as dpool,
            tc.tile_pool(name="mpool", bufs=3) as mpool,
            tc.tile_pool(name="opool", bufs=2) as opool,
            tc.tile_pool(name="fpool", bufs=2) as fpool,
            tc.tile_pool(name="pcpool", bufs=3, space="PSUM") as pcpool,
            tc.tile_pool(name="ptpool", bufs=2, space="PSUM") as ptpool,
            tc.tile_pool(name="potpool", bufs=1, space="PSUM") as potpool,
        ):
            # consts load via the idle Pool queue so the SP queue's first
            # image chunks start immediately
            wpt = consts.tile([128, 5, 128], F16)
            nc.gpsimd.dma_start(out=wpt[:], in_=wp.ap())
            bt = consts.tile([128, 1], F32)
            nc.gpsimd.dma_start(out=bt[:], in_=bias.ap())
            idt = consts.tile([128, 128], F16)
            nc.gpsimd.dma_start(out=idt[:], in_=ident.ap())

            for img in [i for _ in range(reps) for i in range(IMGS_PER_CORE)]:
                # host pre-builds the dup layouts; one full-width (128-
                # partition) DMA per tile is 2x faster than two 64-partition
                # halves (SBUF DMA ports want all 128 partitions)
                # chunked loads: early output tiles only depend on the first
                # chunks, so PE can start before the whole image lands
                NCH = 16
                CW = H * W // NCH
                d1 = dpool.tile([128, H * W], F16, tag="d1")
                dr = dpool.tile([128, H * W], F16, tag="dr")
                for ch in range(NCH):
                    nc.sync.dma_start(
                        out=d1[:, ch * CW : (ch + 1) * CW],
                        in_=x1.ap()[img, :, ch * CW : (ch + 1) * CW],
                    )
                    nc.sync.dma_start(
                        out=dr[:, ch * CW : (ch + 1) * CW],
                        in_=xr.ap()[img, :, ch * CW : (ch + 1) * CW],
                    )
                d1v = d1.rearrange("p (h w) -> p h w", w=W)  # [128, 128, 128]
                drv = dr.rearrange("p (h w) -> p h w", w=W)

                o = opool.tile([128, 128], F16)
                ov = o.rearrange("p (b t) -> p b t", b=4)  # col j = 32*b + t
                # tail tile only fills 2 of its 4 block columns; define the rest
                nc.vector.memset(o[:], 0.0)

                for t, h0 in enumerate(TILE_H0S):
                    Rt = R if t < 31 else 2
                    chunks = CHUNK_STARTS if t < 31 else [0, 124]
                    pc = pcpool.tile([128, Rt * WO], F32, tag="pc")
                    # 3 pairs (kh,0)+(kh,1) via D1
                    for kh in range(3):
                        nc.tensor.matmul(
                            pc[:],
                            lhsT=wpt[:, kh, :],
                            rhs=d1v[:, h0 + kh : h0 + kh + Rt, 0:WO],
                            start=(kh == 0),
                            stop=False,
                        )
                    # single (2,2), upper weight rows zero
                    nc.tensor.matmul(
                        pc[:],
                        lhsT=wpt[:, 4, :],
                        rhs=d1v[:, h0 + 2 : h0 + 2 + Rt, 2 : 2 + WO],
                        start=False,
                        stop=False,
                    )
                    # pair (0,2)+(1,2) via DR (last: DR chunk may land later)
                    nc.tensor.matmul(
                        pc[:],
                        lhsT=wpt[:, 3, :],
                        rhs=drv[:, h0 : h0 + Rt, 2 : 2 + WO],
                        start=False,
                        stop=True,
                    )

                    # tanh(conv + bias) while moving PSUM -> SBUF fp16
                    m = mpool.tile([128, Rt * WO], F16, tag="m")
                    nc.scalar.activation(
                        out=m[:],
                        in_=pc[:],
                        func=mybir.ActivationFunctionType.Tanh,
                        bias=bt[:],
                    )

                    # transpose 128-px chunks: channels -> free dim
                    pt = ptpool.tile([128, len(chunks), 128], F16, tag="pt")
                    for b, cb in enumerate(chunks):
                        nc.tensor.transpose(
                            out=pt[:, b, :], in_=m[:, cb : cb + 128], identity=idt[:]
                        )

                    # channel-min for the tile's chunks -> O[:, 32b + t]
                    nc.vector.tensor_reduce(
                        out=ov[:, 0 : len(chunks), t],
                        in_=pt[:],
                        axis=mybir.AxisListType.X,
                        op=mybir.AluOpType.min,
                    )

                # pixels -> free dim, second tanh, store
                pot = potpool.tile([128, 128], F16)
                nc.tensor.transpose(out=pot[:], in_=o[:], identity=idt[:])
                f = fpool.tile([128, 128], F32)
                nc.scalar.activation(
                    out=f[:], in_=pot[:], func=mybir.ActivationFunctionType.Tanh
                )
                for b, cb in enumerate(CHUNK_STARTS):
                    # main grid: tiles t=0..30, pixel start 504*t + cb
                    nc.sync.dma_start(
                        out=bass.AP(
                            tensor=out,
                            offset=img * NPIX + cb,
                            ap=[[504, 31], [1, 128]],
                        ),
                        in_=f[32 * b : 32 * b + 31, :],
                    )
                for b, cb in enumerate([0, 124]):
                    # tail tile t=31 (rows 124-125): pixel start 124*126 + cb
                    nc.sync.dma_start(
                        out=bass.AP(
                            tensor=out,
                            offset=img * NPIX + 124 * WO + cb,
                            ap=[[504, 1], [1, 128]],
                        ),
                        in_=f[32 * b + 31 : 32 * b + 32, :],
                    )
    nc.compile()
    return nc


def prep_inputs(x, weight, bias):
    """Host-side packing -> per-core input maps (list of 8 dicts)."""
    x = np.asarray(x, dtype=np.float32)
    weight = np.asarray(weight, dtype=np.float32)
    bias = np.asarray(bias, dtype=np.float32)

    x16 = x.astype(np.float16).reshape(N_IMGS, C_IN, H * W)
    # dup layouts: lower half = x, upper half = x shifted by 1 px / 1 row
    x_d1 = np.zeros((N_IMGS, 128, H * W), dtype=np.float16)
    x_d1[:, 0:C_IN, :] = x16
    x_d1[:, C_IN:, : H * W - 1] = x16[:, :, 1:]
    x_dr = np.zeros((N_IMGS, 128, H * W), dtype=np.float16)
    x_dr[:, 0:C_IN, :] = x16
    x_dr[:, C_IN:, : H * W - W] = x16[:, :, W:]

    wp = np.zeros((128, 5, 128), dtype=np.float16)
    # pair slots kh=0..2: rows 0-63 = (kh, kw=0), rows 64-127 = (kh, kw=1)
    for kh in range(3):
        wp[0:64, kh, :] = weight[:, :, kh, 0].T.astype(np.float16)
        wp[64:128, kh, :] = weight[:, :, kh, 1].T.astype(np.float16)
    # slot 3: (0,2) lower + (1,2) upper (row-shifted dup tile)
    wp[0:64, 3, :] = weight[:, :, 0, 2].T.astype(np.float16)
    wp[64:128, 3, :] = weight[:, :, 1, 2].T.astype(np.float16)
    # slot 4: (2,2) lower, upper rows stay zero
    wp[0:64, 4, :] = weight[:, :, 2, 2].T.astype(np.float16)

    b2 = bias.reshape(128, 1).astype(np.float32)
    ident = np.eye(128, dtype=np.float16)

    in_maps = []
    for c in range(N_CORES):
        in_maps.append(
            {
                "x1": np.ascontiguousarray(x_d1[c * IMGS_PER_CORE : (c + 1) * IMGS_PER_CORE]),
                "xr": np.ascontiguousarray(x_dr[c * IMGS_PER_CORE : (c + 1) * IMGS_PER_CORE]),
                "wp": wp,
                "bias": b2,
                "ident": ident,
            }
        )
    return in_maps


def assemble_output(results):
    """results: list of 8 per-core out dicts -> full [32, 1, 126, 126] f32."""
    parts = [np.asarray(results[c]["out"], dtype=np.float32) for c in range(N_CORES)]
    full = np.concatenate(parts, axis=0)  # [32, 15876]
    return full.reshape(N_IMGS, 1, HO, WO)


_NC_CACHE = None


def kernel(x, weight, bias):
    global _NC_CACHE
    if _NC_CACHE is None:
        _NC_CACHE = build_kernel()
    in_maps = prep_inputs(x, weight, bias)
    res = run_bass_kernel_spmd(_NC_CACHE, in_maps, list(range(N_CORES)))
    return assemble_output(res.results)

